# revision 3
# baseline (speedup 1.0000x reference)
"""Trainium2 Bass kernel for per-token fake-quantized Linear:

    y = fake_quant(fake_quant(x) @ W.T + b)      (per-token int8 symmetric)

x: [4, 2048, 4096] f32, W: [4096, 4096] f32, b: [4096] f32.

Strategy (8 NeuronCores, pure data parallel over tokens — zero collectives):
  - 8192 tokens / 8 cores = 1024 tokens per core; W, b replicated.
  - Per-token quantized x values are integers in [-127, 127], which are
    EXACTLY representable in bf16. So the matmul runs on TensorE in bf16
    with integer q as the moving operand and host-pre-packed W.T (bf16) as
    the stationary operand, accumulating in f32 PSUM. The only precision
    loss vs the f32 reference is W's bf16 rounding (~0.1% on y, ~0.6%
    after output re-quant; gate is 2e-2).
  - y = s_x * (q @ Wb.T) + b is recovered with a per-partition ACT scale;
    the bias is folded into the matmul as an extra K=1 rank-1 update
    (b_row^T @ rinv_row), since s_x * rinv_x == 1.
  - Both transposes (q -> q^T for the matmul, z^T -> z for the per-token
    output quant) go through the DMA xbar transpose engine in bf16 via
    small DRAM staging buffers; TensorE does matmul only.

Layout per core (P=128):
  pass 1: per 128-token tile: absmax -> s_x, rinv; q = round(x*rinv) in bf16
          (round-to-nearest-even via the +/- 1.5*2^23 magic constant).
  matmul: out^T accumulation. For each o-group of 512 outputs: psum tiles
          [o128 x t1024] f32 accumulate over 32 k-tiles; lhsT = W^T chunk
          [k128, o128], rhs = q^T strip [k128, t512].
  pass 2: per 128-token tile: z (natural layout) -> y = s_x*z (+b already
          in z), absmax -> s_y, y_q = round(y/s_y)*s_y, DMA out f32.
"""

import sys

if "/opt/trn_rl_repo" not in sys.path:
    sys.path.insert(0, "/opt/trn_rl_repo")

from contextlib import ExitStack

import ml_dtypes
import numpy as np

import concourse.bass as bass
import concourse.mybir as mybir
import concourse.tile as tile
from concourse import bacc
from concourse.bass import ds
from concourse.bass_utils import run_bass_kernel_spmd

N_CORES = 8
P = 128
T = 1024          # tokens per core
K = 4096          # in features
O = 4096          # out features
TT = T // P       # 8 token tiles
KT = K // P       # 32 k tiles
OG = 512          # outputs per o-group (4 o-tiles -> 8 PSUM banks in flight)
NOG = O // OG     # 8 o-groups
OT_PER_G = OG // P  # 4

Q_MAX = 127.0
EPS = 1e-5
MAGIC = 1.5 * 2**23  # f32 add/sub forces round-to-nearest-even to integer
INV_QMAX = float(np.float32(1.0) / np.float32(Q_MAX))

F32 = mybir.dt.float32
BF16 = mybir.dt.bfloat16


def build():
    nc = bacc.Bacc()
    x_ext = nc.declare_dram_parameter("x", [T, K], F32, isOutput=False)
    wt_ext = nc.declare_dram_parameter("wt", [K, O], BF16, isOutput=False)
    b_ext = nc.declare_dram_parameter("b", [O], F32, isOutput=False)
    out_ext = nc.declare_dram_parameter("out", [T, O], F32, isOutput=True)

    with tile.TileContext(nc) as tc, ExitStack() as ctx:
        dram = ctx.enter_context(tc.tile_pool(name="dram", bufs=1, space="DRAM"))
        singles = ctx.enter_context(tc.tile_pool(name="singles", bufs=1))
        xp = ctx.enter_context(tc.tile_pool(name="xp", bufs=2))
        qp = ctx.enter_context(tc.tile_pool(name="qp", bufs=2))
        qt_pool = ctx.enter_context(tc.tile_pool(name="qt", bufs=1))
        sxp = ctx.enter_context(tc.tile_pool(name="sxp", bufs=1))
        stat = ctx.enter_context(tc.tile_pool(name="stat", bufs=2))
        wp = ctx.enter_context(tc.tile_pool(name="wp", bufs=4))
        ztp = ctx.enter_context(tc.tile_pool(name="ztp", bufs=4))
        znp = ctx.enter_context(tc.tile_pool(name="znp", bufs=2))
        yp = ctx.enter_context(tc.tile_pool(name="yp", bufs=2))
        psum = ctx.enter_context(tc.tile_pool(name="psum", bufs=4, space="PSUM"))

        q_dram = dram.tile([T, K], BF16, tag="q_dram")
        zt_dram = dram.tile([O, T], BF16, tag="zt_dram")
        rinv_dram = dram.tile([TT, P], F32, tag="rinv_dram")

        # bias row in bf16 (partition 0), for the K=1 bias matmul
        b_row = singles.tile([1, O], BF16, tag="b_row")
        nc.gpsimd.dma_start(out=b_row, in_=b_ext[:])  # gpsimd DMA casts f32->bf16

        # ---- pass 1: per-token scales + integer quant (natural layout) ----
        sx_tiles = []
        for t in range(TT):
            x_tile = xp.tile([P, K], F32, tag="x_tile")
            nc.scalar.dma_start(out=x_tile, in_=x_ext[ds(t * P, P), :])
            am = stat.tile([P, 1], F32, tag="am_x")
            nc.vector.tensor_reduce(
                out=am, in_=x_tile, axis=mybir.AxisListType.X,
                op=mybir.AluOpType.max, apply_absolute_value=True,
            )
            sx = sxp.tile([P, 1], F32, tag=f"sx{t}")
            # s = max(absmax, EPS) * (1/127)
            nc.vector.tensor_scalar(
                out=sx, in0=am, scalar1=EPS, scalar2=INV_QMAX,
                op0=mybir.AluOpType.max, op1=mybir.AluOpType.mult,
            )
            rinv = stat.tile([P, 1], F32, tag="rinv_x")
            nc.vector.reciprocal(out=rinv, in_=sx)
            nc.scalar.dma_start(out=rinv_dram[t, :], in_=rinv[:, 0:1])
            # r = x * rinv + MAGIC   (in place over x)
            nc.vector.tensor_scalar(
                out=x_tile, in0=x_tile, scalar1=rinv, scalar2=MAGIC,
                op0=mybir.AluOpType.mult, op1=mybir.AluOpType.add,
            )
            # q = r - MAGIC  -> bf16 (exact: |q| <= 127)
            q_tile = qp.tile([P, K], BF16, tag="q_tile")
            nc.vector.tensor_scalar(
                out=q_tile, in0=x_tile, scalar1=MAGIC,
                scalar2=None, op0=mybir.AluOpType.subtract,
            )
            nc.scalar.dma_start(out=q_dram[ds(t * P, P), :], in_=q_tile)
            sx_tiles.append(sx)

        # rinv as a bf16 row vector [1, T] (rhs of the K=1 bias matmul)
        rinv_row = singles.tile([1, T], BF16, tag="rinv_row")
        nc.gpsimd.dma_start(out=rinv_row, in_=rinv_dram[:, :])

        # ---- q^T strips via DMA xbar transpose ----
        qt_tiles = []
        for k in range(KT):
            qt = qt_pool.tile([P, T], BF16, tag=f"qt{k}")
            nc.scalar.dma_start_transpose(qt, q_dram[:, ds(k * P, P)])
            qt_tiles.append(qt)

        # ---- matmul phase: z^T = Wb @ q^T (+ b * rinv row) ----
        for og in range(NOG):
            ps = [
                psum.tile([P, T], F32, tag="ps", name=f"ps_{og}_{i}")
                for i in range(OT_PER_G)
            ]
            for k in range(KT):
                w_tile = wp.tile([P, OG], BF16, tag="w_tile")
                nc.sync.dma_start(
                    out=w_tile, in_=wt_ext[ds(k * P, P), ds(og * OG, OG)]
                )
                for ot in range(OT_PER_G):
                    for th in range(2):
                        nc.tensor.matmul(
                            ps[ot][:, ds(th * 512, 512)],
                            w_tile[:, ds(ot * P, P)],
                            qt_tiles[k][:, ds(th * 512, 512)],
                            start=(k == 0),
                            stop=False,
                        )
            # bias: psum += b_chunk^T @ rinv_row   (K=1 matmul)
            for ot in range(OT_PER_G):
                o0 = og * OG + ot * P
                for th in range(2):
                    nc.tensor.matmul(
                        ps[ot][:, ds(th * 512, 512)],
                        b_row[0:1, ds(o0, P)],
                        rinv_row[0:1, ds(th * 512, 512)],
                        start=False,
                        stop=True,
                    )
            for ot in range(OT_PER_G):
                o0 = og * OG + ot * P
                zt_sb = ztp.tile([P, T], BF16, tag="zt_sb")
                nc.scalar.copy(out=zt_sb, in_=ps[ot])
                nc.sync.dma_start(out=zt_dram[ds(o0, P), :], in_=zt_sb)

        # ---- pass 2: transpose back, scale, requant, store ----
        for t in range(TT):
            z_nat = znp.tile([P, O], BF16, tag="z_nat")
            nc.scalar.dma_start_transpose(z_nat, zt_dram[:, ds(t * P, P)])
            y_tile = yp.tile([P, O], F32, tag="y_tile")
            # y = s_x * z   (bias already inside z)
            nc.scalar.activation(
                out=y_tile, in_=z_nat,
                func=mybir.ActivationFunctionType.Copy, scale=sx_tiles[t],
            )
            am = stat.tile([P, 1], F32, tag="am_y")
            nc.vector.tensor_reduce(
                out=am, in_=y_tile, axis=mybir.AxisListType.X,
                op=mybir.AluOpType.max, apply_absolute_value=True,
            )
            sy = stat.tile([P, 1], F32, tag="sy")
            nc.vector.tensor_scalar(
                out=sy, in0=am, scalar1=EPS, scalar2=INV_QMAX,
                op0=mybir.AluOpType.max, op1=mybir.AluOpType.mult,
            )
            rinvy = stat.tile([P, 1], F32, tag="rinv_y")
            nc.vector.reciprocal(out=rinvy, in_=sy)
            # r = y * rinv_y + MAGIC  (in place)
            nc.vector.tensor_scalar(
                out=y_tile, in0=y_tile, scalar1=rinvy, scalar2=MAGIC,
                op0=mybir.AluOpType.mult, op1=mybir.AluOpType.add,
            )
            # y_q = (r - MAGIC) * s_y  (in place)
            nc.vector.tensor_scalar(
                out=y_tile, in0=y_tile, scalar1=MAGIC, scalar2=sy,
                op0=mybir.AluOpType.subtract, op1=mybir.AluOpType.mult,
            )
            nc.scalar.dma_start(out=out_ext[ds(t * P, P), :], in_=y_tile)

    nc.compile()
    return nc


_NC_CACHE = None


def _get_nc():
    global _NC_CACHE
    if _NC_CACHE is None:
        _NC_CACHE = build()
    return _NC_CACHE


def _run(x, W, b, trace=False):
    nc = _get_nc()
    x2d = np.ascontiguousarray(np.asarray(x, dtype=np.float32).reshape(-1, K))
    wt = np.ascontiguousarray(np.asarray(W, dtype=np.float32).T).astype(
        ml_dtypes.bfloat16
    )
    bf = np.ascontiguousarray(np.asarray(b, dtype=np.float32))
    in_maps = [
        {"x": np.ascontiguousarray(x2d[i * T:(i + 1) * T]), "wt": wt, "b": bf}
        for i in range(N_CORES)
    ]
    res = run_bass_kernel_spmd(nc, in_maps, list(range(N_CORES)), trace=trace)
    out = np.concatenate([res.results[i]["out"] for i in range(N_CORES)], axis=0)
    return out, res


def kernel(x, W, b):
    out, _ = _run(x, W, b, trace=False)
    return out.reshape(np.asarray(x).shape[:-1] + (O,)).astype(np.float32)
